# revision 17
# baseline (speedup 1.0000x reference)
"""Trainium2 Bass kernel for nn_CPL_MoE (query-guided MoE: gating MLP -> top-2
softmax gates -> gated expert matmul accumulation + gauss head + balance loss).

Strategy: vocab/tensor-parallel across the 8 NeuronCores (each core owns a
V/8 = 1500 column slice of comp_w / comp_b / comp output; gating replicated).
The big expert matmuls run in float32r (full-rate fp32 on the PE array) and
exploit the top-2 routing sparsity: routing is computed on the host from the
gating inputs, and the program only emits matmul work for (expert, sample)
pairs that the top-2 selection actually activates.  The gate VALUES applied on
device come from the on-device gating computation, so results match the dense
computation exactly (skipped blocks would have been multiplied by gates == 0).

Self-contained: only needs numpy + the concourse (Bass) stack available in the
container.
"""

import os
import numpy as np
from contextlib import ExitStack

import concourse.bass as bass
import concourse.tile as tile
import concourse.bacc as bacc
from concourse import mybir, bass_utils

F32 = mybir.dt.float32
F32R = mybir.dt.float32r
AX = mybir.AxisListType
ALU = mybir.AluOpType
ACTF = mybir.ActivationFunctionType

# problem shapes (hardcoded per spec)
B, L, H, E, V = 32, 64, 1024, 8, 12000
OUT = 16
HID = 512
COEF = 0.1
NCORES = 8
VS = V // NCORES            # 1500 vocab columns per core
BL = B * L                  # 2048
HC = H // 128               # 8 contraction chunks
NBLC = BL // 128            # 16 row tiles of comp
VCS = [(0, 512), (512, 512), (1024, VS - 1024)]   # psum-bank sized v chunks

_cache: dict = {}


def _dense_work():
    tiles = tuple((blc, True, True) for blc in range(B // 2))
    return tuple(tiles for _ in range(E))


def _route_work(top2):
    """top2: [B, 2] expert ids per sample -> per-expert (blc, drain0, drain1) tiles.

    A row tile blc covers samples (2*blc, 2*blc+1).  An expert computes a tile
    when either sample routes to it; only the active halves are drained into
    the accumulator (the other half's gate is 0 anyway)."""
    sel = [set() for _ in range(E)]
    for s in range(B):
        for e in top2[s]:
            sel[int(e)].add(s)
    work = []
    for e in range(E):
        tiles = []
        for blc in range(B // 2):
            d0 = 2 * blc in sel[e]
            d1 = 2 * blc + 1 in sel[e]
            if d0 or d1:
                tiles.append((blc, d0, d1))
        work.append(tuple(tiles))
    return tuple(work)


def _build(work, with_bias):
    nc = bacc.Bacc("TRN2", target_bir_lowering=False, debug=False)

    # ---- DRAM I/O ----
    hT_d = nc.dram_tensor("hT_d", [H, BL], F32, kind="ExternalInput")
    qT_d = nc.dram_tensor("qT_d", [H, B], F32, kind="ExternalInput")
    hlT_d = nc.dram_tensor("hlT_d", [H, B], F32, kind="ExternalInput")
    w1_d = nc.dram_tensor("w1_d", [H, HID], F32, kind="ExternalInput")
    b1_d = nc.dram_tensor("b1_d", [HID], F32, kind="ExternalInput")
    w2_d = nc.dram_tensor("w2_d", [HID, E], F32, kind="ExternalInput")
    b2_d = nc.dram_tensor("b2_d", [E], F32, kind="ExternalInput")
    gwT_d = nc.dram_tensor("gwT_d", [H, E * OUT], F32, kind="ExternalInput")
    gb_d = nc.dram_tensor("gb_d", [E * OUT], F32, kind="ExternalInput")
    wt_d = nc.dram_tensor("wt_d", [E, H, VS], F32, kind="ExternalInput")
    ones_d = nc.dram_tensor("ones_d", [128], F32, kind="ExternalInput")
    cb_d = nc.dram_tensor("cb_d", [E, VS], F32, kind="ExternalInput")

    comp_d = nc.dram_tensor("comp_d", [BL, VS], F32, kind="ExternalOutput")
    gp_d = nc.dram_tensor("gp_d", [B, OUT], F32, kind="ExternalOutput")
    loss_d = nc.dram_tensor("loss_d", [1, 1], F32, kind="ExternalOutput")
    gsc_d = nc.dram_tensor("gsc_d", [B, E], F32, kind="Internal")

    with tile.TileContext(nc) as tc, ExitStack() as ctx:
        misc = ctx.enter_context(tc.tile_pool(name="misc", bufs=1))
        accp = ctx.enter_context(tc.tile_pool(name="accp", bufs=NBLC))
        wpool = ctx.enter_context(tc.tile_pool(name="wpool", bufs=13))
        w1pool = ctx.enter_context(tc.tile_pool(name="w1pool", bufs=2))
        cbpool = ctx.enter_context(tc.tile_pool(name="cbpool", bufs=2))
        ps = ctx.enter_context(tc.tile_pool(name="ps", bufs=8, space="PSUM"))

        def mtile(shape, name):
            return misc.tile(shape, F32, name=name, tag=name)

        # ---- constants ----
        ones = mtile([1, 128], "ones")
        nc.vector.memset(ones[:], 1.0)
        onesr = misc.tile([1, 128], F32R, name="onesr", tag="onesr")
        nc.gpsimd.dma_start(onesr[0:1, :], bass.AP(ones_d, 0, [[1, 1], [1, 128]]).bitcast(F32R))
        onescol = mtile([32, 1], "onescol")
        nc.vector.memset(onescol[:], 1.0)

        # ---- gating input DMAs ----
        qT = mtile([128, HC * B], "qT")            # [p, hc*32+b]
        nc.gpsimd.dma_start(qT[:], bass.AP(qT_d, 0, [[B, 128], [128 * B, HC], [1, B]]))
        hlT = mtile([128, HC * B], "hlT")
        nc.gpsimd.dma_start(hlT[:], bass.AP(hlT_d, 0, [[B, 128], [128 * B, HC], [1, B]]))
        b1sb = mtile([128, 4], "b1sb")             # [p, hidc]
        nc.gpsimd.dma_start(b1sb[:], bass.AP(b1_d, 0, [[1, 128], [128, 4]]))
        w2sb = mtile([128, 4 * E], "w2sb")         # [p, hidc*8+e]
        nc.gpsimd.dma_start(w2sb[:], bass.AP(w2_d, 0, [[E, 128], [128 * E, 4], [1, E]]))
        b2sb = mtile([1, E], "b2sb")
        nc.gpsimd.dma_start(b2sb[:], bass.AP(b2_d, 0, [[1, 1], [1, E]]))
        gbsb = mtile([1, E * OUT], "gbsb")
        nc.gpsimd.dma_start(gbsb[:], bass.AP(gb_d, 0, [[1, 1], [1, E * OUT]]))

        # ---- gating MLP: hidT [hid(4x128 part chunks), b] = relu(w1.T @ q + b1) ----
        hidT = mtile([128, 4 * B], "hidT")         # [p, hidc*32+b]
        hps = [ps.tile([128, B], F32, name=f"hps{i}", tag="ps") for i in range(4)]
        for hc in range(HC):
            w1c = w1pool.tile([128, HID], F32, name=f"w1c{hc}", tag="w1c")
            nc.gpsimd.dma_start(
                w1c[:], bass.AP(w1_d, hc * 128 * HID, [[HID, 128], [1, HID]])
            )
            for hidc in range(4):
                nc.tensor.matmul(
                    hps[hidc][:],
                    w1c[:, hidc * 128:(hidc + 1) * 128],
                    qT[:, hc * B:(hc + 1) * B],
                    start=(hc == 0),
                    stop=(hc == HC - 1),
                )
        for hidc in range(4):
            nc.scalar.activation(
                hidT[:, hidc * B:(hidc + 1) * B], hps[hidc][:], ACTF.Relu,
                bias=b1sb[:, hidc:hidc + 1], scale=1.0,
            )

        # ---- logits [b=32, e=8] ----
        lps = ps.tile([B, E], F32, name="lps", tag="ps")
        for hidc in range(4):
            nc.tensor.matmul(
                lps[:], hidT[:, hidc * B:(hidc + 1) * B], w2sb[:, hidc * E:(hidc + 1) * E],
                start=(hidc == 0), stop=False,
            )
        nc.tensor.matmul(lps[:], ones[0:1, 0:B], b2sb[0:1, :], start=False, stop=True)
        lsb = mtile([B, E], "lsb")
        nc.scalar.copy(lsb[:], lps[:])

        # ---- top-2 softmax gates [32, 8] ----
        m1 = mtile([B, 1], "m1")
        negm1 = mtile([B, 1], "negm1")
        e1 = mtile([B, E], "e1")
        mask1 = mtile([B, E], "mask1")
        lmsk = mtile([B, E], "lmsk")
        m2 = mtile([B, 1], "m2")
        mask2 = mtile([B, E], "mask2")
        e2m = mtile([B, 1], "e2m")
        den = mtile([B, 1], "den")
        rec = mtile([B, 1], "rec")
        gates = mtile([B, E], "gates")
        nc.vector.tensor_reduce(m1[:], lsb[:], axis=AX.X, op=ALU.max)
        nc.vector.tensor_scalar_mul(negm1[:], m1[:], -1.0)
        nc.scalar.activation(e1[:], lsb[:], ACTF.Exp, bias=negm1[:], scale=1.0)
        nc.vector.tensor_scalar(mask1[:], lsb[:], m1[:], None, op0=ALU.is_ge)
        nc.vector.scalar_tensor_tensor(lmsk[:], mask1[:], -1e30, lsb[:], op0=ALU.mult, op1=ALU.add)
        nc.vector.tensor_reduce(m2[:], lmsk[:], axis=AX.X, op=ALU.max)
        nc.vector.tensor_scalar(mask2[:], lsb[:], m2[:], None, op0=ALU.is_ge)
        nc.scalar.activation(e2m[:], m2[:], ACTF.Exp, bias=negm1[:], scale=1.0)
        nc.vector.tensor_scalar_add(den[:], e2m[:], 1.0)
        nc.vector.reciprocal(rec[:], den[:])
        nc.vector.scalar_tensor_tensor(gates[:], e1[:], rec[:], mask2[:], op0=ALU.mult, op1=ALU.mult)

        # ---- replicate gates along 64 l-rows: grep[p, blc*8+e] = gates[2blc+p//64, e] ----
        grep = mtile([128, NBLC * E], "grep")
        nc.gpsimd.dma_start(gsc_d.ap(), gates[:])
        for phh in range(2):
            nc.gpsimd.dma_start(
                grep[phh * 64:(phh + 1) * 64, :],
                bass.AP(gsc_d, phh * E, [[0, 64], [2 * E, NBLC], [1, E]]),
            )

        # ---- moe balance loss ----
        imp_ps = ps.tile([E, 1], F32, name="imp_ps", tag="ps")
        nc.tensor.matmul(imp_ps[:], gates[:], onescol[:, :], start=True, stop=True)
        imp = mtile([E, 1], "imp")
        nc.scalar.copy(imp[:], imp_ps[:])
        s1_ps = ps.tile([1, 1], F32, name="s1_ps", tag="ps")
        nc.tensor.matmul(s1_ps[:], imp[:], onescol[0:E, :], start=True, stop=True)
        s2_ps = ps.tile([1, 1], F32, name="s2_ps", tag="ps")
        nc.tensor.matmul(s2_ps[:], imp[:], imp[:], start=True, stop=True)
        s1 = mtile([1, 1], "s1")
        s2 = mtile([1, 1], "s2")
        nc.scalar.copy(s1[:], s1_ps[:])
        nc.scalar.copy(s2[:], s2_ps[:])
        mean = mtile([1, 1], "mean")
        nc.scalar.mul(mean[:], s1[:], 1.0 / E)
        meps = mtile([1, 1], "meps")
        nc.vector.tensor_scalar_add(meps[:], mean[:], 1e-10)
        rmean = mtile([1, 1], "rmean")
        nc.vector.reciprocal(rmean[:], meps[:])
        s1sq = mtile([1, 1], "s1sq")
        nc.vector.tensor_mul(s1sq[:], s1[:], s1[:])
        va = mtile([1, 1], "va")
        nc.scalar.mul(va[:], s2[:], 1.0 / (E - 1))
        vb = mtile([1, 1], "vb")
        nc.scalar.mul(vb[:], s1sq[:], 1.0 / (E * (E - 1)))
        var = mtile([1, 1], "var")
        nc.vector.tensor_sub(var[:], va[:], vb[:])
        std = mtile([1, 1], "std")
        nc.scalar.sqrt(std[:], var[:])
        lossv = mtile([1, 1], "lossv")
        nc.vector.tensor_mul(lossv[:], std[:], rmean[:])
        losso = mtile([1, 1], "losso")
        nc.scalar.mul(losso[:], lossv[:], COEF)
        nc.gpsimd.dma_start(loss_d.ap(), losso[:])

        # ---- gauss head: gp = sigmoid(sum_e g[b,e] * (h_last @ gauss_w[e].T + gauss_b[e])) ----
        gps_ = ps.tile([B, E * OUT], F32, name="gps_", tag="ps")
        for half in range(2):
            gwc = w1pool.tile([128, 4 * E * OUT], F32, name=f"gwc{half}", tag="w1c")
            nc.gpsimd.dma_start(
                gwc[:],
                bass.AP(
                    gwT_d, half * 4 * 128 * E * OUT,
                    [[E * OUT, 128], [128 * E * OUT, 4], [1, E * OUT]],
                ),
            )
            for k in range(4):
                hc = half * 4 + k
                nc.tensor.matmul(
                    gps_[:], hlT[:, hc * B:(hc + 1) * B], gwc[:, k * E * OUT:(k + 1) * E * OUT],
                    start=(hc == 0), stop=False,
                )
        nc.tensor.matmul(gps_[:], ones[0:1, 0:B], gbsb[0:1, :], start=False, stop=True)
        gacc = mtile([B, OUT], "gacc")
        nc.vector.tensor_scalar(gacc[:], gps_[:, 0:OUT], gates[:, 0:1], None, op0=ALU.mult)
        for e in range(1, E):
            nc.vector.scalar_tensor_tensor(
                gacc[:], gps_[:, e * OUT:(e + 1) * OUT], gates[:, e:e + 1], gacc[:],
                op0=ALU.mult, op1=ALU.add,
            )
        gpo = mtile([B, OUT], "gpo")
        nc.scalar.activation(gpo[:], gacc[:], ACTF.Sigmoid)
        nc.gpsimd.dma_start(gp_d.ap(), gpo[:])

        # ---- h^T resident [p, hc*2048 + bl]: 16 partition-half DMAs across queues ----
        hT = misc.tile([128, HC * BL], F32R, name="hT", tag="hT")
        for hc in range(HC):
            for phalf in range(2):
                nc.sync.dma_start(
                    hT[phalf * 64:(phalf + 1) * 64, hc * BL:(hc + 1) * BL],
                    bass.AP(
                        hT_d, (hc * 128 + phalf * 64) * BL, [[BL, 64], [1, BL]]
                    ).bitcast(F32R),
                )

        # ---- acc tiles ----
        accs = [accp.tile([128, VS], F32, name=f"acc{i}", tag="acc") for i in range(NBLC)]


        # ---- main gated expert matmuls, two W-streaming passes over vocab ----
        written = set()
        for ci, (off, n) in enumerate(VCS):
            subs = [(so, min(512, n - so)) for so in range(0, n, 512)]
            for e in range(E):
                tiles = work[e]
                if not tiles:
                    continue
                wts = []
                for hc in range(HC):
                    wt = wpool.tile([128, 512], F32R, name=f"wt{ci}_{e}_{hc}", tag="wt")
                    nc.sync.dma_start(
                        wt[:, 0:n],
                        bass.AP(
                            wt_d, e * H * VS + hc * 128 * VS + off, [[VS, 128], [1, n]]
                        ).bitcast(F32R),
                    )
                    wts.append(wt)
                if with_bias:
                    cbt = cbpool.tile([1, 512], F32R, name=f"cbt{ci}_{e}", tag="cbt")
                    nc.sync.dma_start(
                        cbt[0:1, 0:n],
                        bass.AP(cb_d, e * VS + off, [[1, 1], [1, n]]).bitcast(F32R),
                    )
                for (blc, d0, d1) in tiles:
                    for (so, sn) in subs:
                        pt = ps.tile([128, 512], F32, name=f"pt{ci}_{e}_{blc}_{so}", tag="ps")
                        for hc in range(HC):
                            nc.tensor.matmul(
                                pt[:, 0:sn],
                                hT[:, hc * BL + blc * 128: hc * BL + blc * 128 + 128],
                                wts[hc][:, so:so + sn],
                                start=(hc == 0),
                                stop=(hc == HC - 1 and not with_bias),
                            )
                        if with_bias:
                            nc.tensor.matmul(
                                pt[:, 0:sn],
                                onesr[0:1, 0:128],
                                cbt[0:1, so:so + sn],
                                start=False,
                                stop=True,
                            )
                        for phh, active in ((0, d0), (1, d1)):
                            if not active:
                                continue
                            s = 2 * blc + phh
                            psrc = pt[phh * 64:(phh + 1) * 64, 0:sn]
                            gsc = grep[phh * 64:(phh + 1) * 64, blc * E + e: blc * E + e + 1]
                            adst = accs[blc][phh * 64:(phh + 1) * 64, off + so:off + so + sn]
                            if (s, off + so) in written:
                                nc.vector.scalar_tensor_tensor(
                                    adst, psrc, gsc, adst, op0=ALU.mult, op1=ALU.add
                                )
                            else:
                                nc.scalar.activation(adst, psrc, ACTF.Copy, scale=gsc)
                                written.add((s, off + so))
            for blc in range(NBLC):
                nc.scalar.dma_start(
                    comp_d.ap()[blc * 128:(blc + 1) * 128, off:off + n],
                    accs[blc][:, off:off + n],
                )

    nc.compile()
    return nc


def _get_program(work, with_bias):
    key = (work, with_bias)
    if key not in _cache:
        _cache[key] = _build(work, with_bias)
    return _cache[key]


def kernel(**inputs):
    f = lambda k: np.ascontiguousarray(np.asarray(inputs[k], dtype=np.float32))
    q = f("query_repr")
    h = f("h")
    w1 = f("gate_w1")
    b1 = f("gate_b1")
    w2 = f("gate_w2")
    b2 = f("gate_b2")
    gw = f("gauss_w")
    gb = f("gauss_b")
    cw = f("comp_w")
    cb = f("comp_b")

    # host routing (work-list only; gate values come from the device)
    hid = np.maximum(q @ w1 + b1, 0.0)
    logits = hid @ w2 + b2
    order = np.argsort(-logits, axis=1, kind="stable")
    srt = np.take_along_axis(logits, order, axis=1)
    tie_risk = np.min(srt[:, 1] - srt[:, 2]) < 1e-5 or np.min(srt[:, 0] - srt[:, 1]) < 1e-5
    mode = os.environ.get("MOE_KERNEL_MODE", "auto")
    if mode == "dense" or (mode == "auto" and tie_risk):
        work = _dense_work()
    else:
        work = _route_work(order[:, :2])
    with_bias = bool(np.any(cb != 0.0))

    nc = _get_program(work, with_bias)

    hT = np.ascontiguousarray(h.reshape(BL, H).T)
    shared = {
        "hT_d": hT,
        "qT_d": np.ascontiguousarray(q.T),
        "hlT_d": np.ascontiguousarray(h[:, -1].T),
        "w1_d": w1,
        "b1_d": b1,
        "w2_d": w2,
        "b2_d": b2,
        "gwT_d": np.ascontiguousarray(gw.transpose(2, 0, 1).reshape(H, E * OUT)),
        "gb_d": gb.reshape(E * OUT),
        "ones_d": np.ones(128, np.float32),
    }
    in_maps = []
    for c in range(NCORES):
        sl = slice(c * VS, (c + 1) * VS)
        in_maps.append(
            dict(
                shared,
                wt_d=np.ascontiguousarray(cw[:, sl, :].transpose(0, 2, 1)),
                cb_d=np.ascontiguousarray(cb[:, sl]),
            )
        )

    trace = os.environ.get("MOE_KERNEL_TRACE") == "1"
    if trace:
        try:
            import sys, types

            if "antenv.axon_hooks" not in sys.modules:
                import antenv  # noqa: F401

                mod = types.ModuleType("antenv.axon_hooks")
                mod._hook = None
                mod.set_axon_ntff_profile_hook = lambda hk: setattr(mod, "_hook", hk)
                mod.get_axon_ntff_profile_hook = lambda: mod._hook
                sys.modules["antenv.axon_hooks"] = mod
                from trn_agent_boot.trn_boot import _ntff_profile_via_ctypes

                mod._hook = _ntff_profile_via_ctypes("/opt/axon/libaxon_pjrt.so")
        except Exception as exc:  # pragma: no cover
            print(f"trace hook install failed: {exc}")

    res = bass_utils.run_bass_kernel_spmd(
        nc, in_maps, core_ids=list(range(NCORES)), trace=trace
    )
    if trace and res.exec_time_ns is not None:
        print(f"HW exec time: {res.exec_time_ns} ns")

    comp = np.concatenate(
        [res.results[c]["comp_d"].reshape(B, L, VS) for c in range(NCORES)], axis=2
    )
    gp = res.results[0]["gp_d"]
    loss = np.float32(res.results[0]["loss_d"][0, 0])
    return gp, comp, loss


# revision 18
# speedup vs baseline: 1.0237x; 1.0237x over previous
"""Trainium2 Bass kernel for nn_CPL_MoE (query-guided MoE: gating MLP -> top-2
softmax gates -> gated expert matmul accumulation + gauss head + balance loss).

Strategy: vocab/tensor-parallel across the 8 NeuronCores (each core owns a
V/8 = 1500 column slice of comp_w / comp_b / comp output; gating replicated).
The big expert matmuls run in float32r (full-rate fp32 on the PE array) and
exploit the top-2 routing sparsity: routing is computed on the host from the
gating inputs, and the program only emits matmul work for (expert, sample)
pairs that the top-2 selection actually activates.  The gate VALUES applied on
device come from the on-device gating computation, so results match the dense
computation exactly (skipped blocks would have been multiplied by gates == 0).

Self-contained: only needs numpy + the concourse (Bass) stack available in the
container.
"""

import os
import numpy as np
from contextlib import ExitStack

import concourse.bass as bass
import concourse.tile as tile
import concourse.bacc as bacc
from concourse import mybir, bass_utils

F32 = mybir.dt.float32
F32R = mybir.dt.float32r
AX = mybir.AxisListType
ALU = mybir.AluOpType
ACTF = mybir.ActivationFunctionType

# problem shapes (hardcoded per spec)
B, L, H, E, V = 32, 64, 1024, 8, 12000
OUT = 16
HID = 512
COEF = 0.1
NCORES = 8
VS = V // NCORES            # 1500 vocab columns per core
BL = B * L                  # 2048
HC = H // 128               # 8 contraction chunks
NBLC = BL // 128            # 16 row tiles of comp
VCS = [(0, 512), (512, 512), (1024, VS - 1024)]   # psum-bank sized v chunks

_cache: dict = {}


def _dense_work():
    tiles = tuple((blc, True, True) for blc in range(B // 2))
    return tuple(tiles for _ in range(E))


def _route_work(top2):
    """top2: [B, 2] expert ids per sample -> per-expert (blc, drain0, drain1) tiles.

    A row tile blc covers samples (2*blc, 2*blc+1).  An expert computes a tile
    when either sample routes to it; only the active halves are drained into
    the accumulator (the other half's gate is 0 anyway)."""
    sel = [set() for _ in range(E)]
    for s in range(B):
        for e in top2[s]:
            sel[int(e)].add(s)
    work = []
    for e in range(E):
        tiles = []
        for blc in range(B // 2):
            d0 = 2 * blc in sel[e]
            d1 = 2 * blc + 1 in sel[e]
            if d0 or d1:
                tiles.append((blc, d0, d1))
        work.append(tuple(tiles))
    return tuple(work)


def _build(work, with_bias):
    nc = bacc.Bacc("TRN2", target_bir_lowering=False, debug=False)

    # ---- DRAM I/O ----
    hT_d = nc.dram_tensor("hT_d", [H, BL], F32, kind="ExternalInput")
    qT_d = nc.dram_tensor("qT_d", [H, B], F32, kind="ExternalInput")
    hlT_d = nc.dram_tensor("hlT_d", [H, B], F32, kind="ExternalInput")
    w1_d = nc.dram_tensor("w1_d", [H, HID], F32, kind="ExternalInput")
    b1_d = nc.dram_tensor("b1_d", [HID], F32, kind="ExternalInput")
    w2_d = nc.dram_tensor("w2_d", [HID, E], F32, kind="ExternalInput")
    b2_d = nc.dram_tensor("b2_d", [E], F32, kind="ExternalInput")
    gwT_d = nc.dram_tensor("gwT_d", [H, E * OUT], F32, kind="ExternalInput")
    gb_d = nc.dram_tensor("gb_d", [E * OUT], F32, kind="ExternalInput")
    wt_d = nc.dram_tensor("wt_d", [E, H, VS], F32, kind="ExternalInput")
    ones_d = nc.dram_tensor("ones_d", [128], F32, kind="ExternalInput")
    cb_d = nc.dram_tensor("cb_d", [E, VS], F32, kind="ExternalInput")

    comp_d = nc.dram_tensor("comp_d", [BL, VS], F32, kind="ExternalOutput")
    gp_d = nc.dram_tensor("gp_d", [B, OUT], F32, kind="ExternalOutput")
    loss_d = nc.dram_tensor("loss_d", [1, 1], F32, kind="ExternalOutput")
    gsc_d = nc.dram_tensor("gsc_d", [B, E], F32, kind="Internal")

    with tile.TileContext(nc) as tc, ExitStack() as ctx:
        misc = ctx.enter_context(tc.tile_pool(name="misc", bufs=1))
        accp = ctx.enter_context(tc.tile_pool(name="accp", bufs=NBLC))
        wpool = ctx.enter_context(tc.tile_pool(name="wpool", bufs=13))
        w1pool = ctx.enter_context(tc.tile_pool(name="w1pool", bufs=2))
        cbpool = ctx.enter_context(tc.tile_pool(name="cbpool", bufs=2))
        ps = ctx.enter_context(tc.tile_pool(name="ps", bufs=8, space="PSUM"))

        def mtile(shape, name):
            return misc.tile(shape, F32, name=name, tag=name)

        # ---- constants ----
        ones = mtile([1, 128], "ones")
        nc.vector.memset(ones[:], 1.0)
        onesr = misc.tile([1, 128], F32R, name="onesr", tag="onesr")
        nc.sync.dma_start(onesr[0:1, :], bass.AP(ones_d, 0, [[1, 1], [1, 128]]).bitcast(F32R))
        onescol = mtile([32, 1], "onescol")
        nc.vector.memset(onescol[:], 1.0)

        # ---- gating input DMAs ----
        qT = mtile([128, HC * B], "qT")            # [p, hc*32+b]
        nc.sync.dma_start(qT[:], bass.AP(qT_d, 0, [[B, 128], [128 * B, HC], [1, B]]))
        hlT = mtile([128, HC * B], "hlT")
        nc.sync.dma_start(hlT[:], bass.AP(hlT_d, 0, [[B, 128], [128 * B, HC], [1, B]]))
        b1sb = mtile([128, 4], "b1sb")             # [p, hidc]
        nc.sync.dma_start(b1sb[:], bass.AP(b1_d, 0, [[1, 128], [128, 4]]))
        w2sb = mtile([128, 4 * E], "w2sb")         # [p, hidc*8+e]
        nc.sync.dma_start(w2sb[:], bass.AP(w2_d, 0, [[E, 128], [128 * E, 4], [1, E]]))
        b2sb = mtile([1, E], "b2sb")
        nc.sync.dma_start(b2sb[:], bass.AP(b2_d, 0, [[1, 1], [1, E]]))
        gbsb = mtile([1, E * OUT], "gbsb")
        nc.sync.dma_start(gbsb[:], bass.AP(gb_d, 0, [[1, 1], [1, E * OUT]]))

        # ---- gating MLP: hidT [hid(4x128 part chunks), b] = relu(w1.T @ q + b1) ----
        hidT = mtile([128, 4 * B], "hidT")         # [p, hidc*32+b]
        hps = [ps.tile([128, B], F32, name=f"hps{i}", tag="ps") for i in range(4)]
        for hc in range(HC):
            w1c = w1pool.tile([128, HID], F32, name=f"w1c{hc}", tag="w1c")
            nc.gpsimd.dma_start(
                w1c[:], bass.AP(w1_d, hc * 128 * HID, [[HID, 128], [1, HID]])
            )
            for hidc in range(4):
                nc.tensor.matmul(
                    hps[hidc][:],
                    w1c[:, hidc * 128:(hidc + 1) * 128],
                    qT[:, hc * B:(hc + 1) * B],
                    start=(hc == 0),
                    stop=(hc == HC - 1),
                )
        for hidc in range(4):
            nc.scalar.activation(
                hidT[:, hidc * B:(hidc + 1) * B], hps[hidc][:], ACTF.Relu,
                bias=b1sb[:, hidc:hidc + 1], scale=1.0,
            )

        # ---- logits [b=32, e=8] ----
        lps = ps.tile([B, E], F32, name="lps", tag="ps")
        for hidc in range(4):
            nc.tensor.matmul(
                lps[:], hidT[:, hidc * B:(hidc + 1) * B], w2sb[:, hidc * E:(hidc + 1) * E],
                start=(hidc == 0), stop=False,
            )
        nc.tensor.matmul(lps[:], ones[0:1, 0:B], b2sb[0:1, :], start=False, stop=True)
        lsb = mtile([B, E], "lsb")
        nc.scalar.copy(lsb[:], lps[:])

        # ---- top-2 softmax gates [32, 8] ----
        m1 = mtile([B, 1], "m1")
        negm1 = mtile([B, 1], "negm1")
        e1 = mtile([B, E], "e1")
        mask1 = mtile([B, E], "mask1")
        lmsk = mtile([B, E], "lmsk")
        m2 = mtile([B, 1], "m2")
        mask2 = mtile([B, E], "mask2")
        e2m = mtile([B, 1], "e2m")
        den = mtile([B, 1], "den")
        rec = mtile([B, 1], "rec")
        gates = mtile([B, E], "gates")
        nc.vector.tensor_reduce(m1[:], lsb[:], axis=AX.X, op=ALU.max)
        nc.vector.tensor_scalar_mul(negm1[:], m1[:], -1.0)
        nc.scalar.activation(e1[:], lsb[:], ACTF.Exp, bias=negm1[:], scale=1.0)
        nc.vector.tensor_scalar(mask1[:], lsb[:], m1[:], None, op0=ALU.is_ge)
        nc.vector.scalar_tensor_tensor(lmsk[:], mask1[:], -1e30, lsb[:], op0=ALU.mult, op1=ALU.add)
        nc.vector.tensor_reduce(m2[:], lmsk[:], axis=AX.X, op=ALU.max)
        nc.vector.tensor_scalar(mask2[:], lsb[:], m2[:], None, op0=ALU.is_ge)
        nc.scalar.activation(e2m[:], m2[:], ACTF.Exp, bias=negm1[:], scale=1.0)
        nc.vector.tensor_scalar_add(den[:], e2m[:], 1.0)
        nc.vector.reciprocal(rec[:], den[:])
        nc.vector.scalar_tensor_tensor(gates[:], e1[:], rec[:], mask2[:], op0=ALU.mult, op1=ALU.mult)

        # ---- replicate gates along 64 l-rows: grep[p, blc*8+e] = gates[2blc+p//64, e] ----
        grep = mtile([128, NBLC * E], "grep")
        nc.gpsimd.dma_start(gsc_d.ap(), gates[:])
        for phh in range(2):
            nc.gpsimd.dma_start(
                grep[phh * 64:(phh + 1) * 64, :],
                bass.AP(gsc_d, phh * E, [[0, 64], [2 * E, NBLC], [1, E]]),
            )

        # ---- moe balance loss ----
        imp_ps = ps.tile([E, 1], F32, name="imp_ps", tag="ps")
        nc.tensor.matmul(imp_ps[:], gates[:], onescol[:, :], start=True, stop=True)
        imp = mtile([E, 1], "imp")
        nc.scalar.copy(imp[:], imp_ps[:])
        s1_ps = ps.tile([1, 1], F32, name="s1_ps", tag="ps")
        nc.tensor.matmul(s1_ps[:], imp[:], onescol[0:E, :], start=True, stop=True)
        s2_ps = ps.tile([1, 1], F32, name="s2_ps", tag="ps")
        nc.tensor.matmul(s2_ps[:], imp[:], imp[:], start=True, stop=True)
        s1 = mtile([1, 1], "s1")
        s2 = mtile([1, 1], "s2")
        nc.scalar.copy(s1[:], s1_ps[:])
        nc.scalar.copy(s2[:], s2_ps[:])
        mean = mtile([1, 1], "mean")
        nc.scalar.mul(mean[:], s1[:], 1.0 / E)
        meps = mtile([1, 1], "meps")
        nc.vector.tensor_scalar_add(meps[:], mean[:], 1e-10)
        rmean = mtile([1, 1], "rmean")
        nc.vector.reciprocal(rmean[:], meps[:])
        s1sq = mtile([1, 1], "s1sq")
        nc.vector.tensor_mul(s1sq[:], s1[:], s1[:])
        va = mtile([1, 1], "va")
        nc.scalar.mul(va[:], s2[:], 1.0 / (E - 1))
        vb = mtile([1, 1], "vb")
        nc.scalar.mul(vb[:], s1sq[:], 1.0 / (E * (E - 1)))
        var = mtile([1, 1], "var")
        nc.vector.tensor_sub(var[:], va[:], vb[:])
        std = mtile([1, 1], "std")
        nc.scalar.sqrt(std[:], var[:])
        lossv = mtile([1, 1], "lossv")
        nc.vector.tensor_mul(lossv[:], std[:], rmean[:])
        losso = mtile([1, 1], "losso")
        nc.scalar.mul(losso[:], lossv[:], COEF)
        nc.gpsimd.dma_start(loss_d.ap(), losso[:])

        # ---- gauss head: gp = sigmoid(sum_e g[b,e] * (h_last @ gauss_w[e].T + gauss_b[e])) ----
        gps_ = ps.tile([B, E * OUT], F32, name="gps_", tag="ps")
        for half in range(2):
            gwc = w1pool.tile([128, 4 * E * OUT], F32, name=f"gwc{half}", tag="w1c")
            nc.gpsimd.dma_start(
                gwc[:],
                bass.AP(
                    gwT_d, half * 4 * 128 * E * OUT,
                    [[E * OUT, 128], [128 * E * OUT, 4], [1, E * OUT]],
                ),
            )
            for k in range(4):
                hc = half * 4 + k
                nc.tensor.matmul(
                    gps_[:], hlT[:, hc * B:(hc + 1) * B], gwc[:, k * E * OUT:(k + 1) * E * OUT],
                    start=(hc == 0), stop=False,
                )
        nc.tensor.matmul(gps_[:], ones[0:1, 0:B], gbsb[0:1, :], start=False, stop=True)
        gacc = mtile([B, OUT], "gacc")
        nc.vector.tensor_scalar(gacc[:], gps_[:, 0:OUT], gates[:, 0:1], None, op0=ALU.mult)
        for e in range(1, E):
            nc.vector.scalar_tensor_tensor(
                gacc[:], gps_[:, e * OUT:(e + 1) * OUT], gates[:, e:e + 1], gacc[:],
                op0=ALU.mult, op1=ALU.add,
            )
        gpo = mtile([B, OUT], "gpo")
        nc.scalar.activation(gpo[:], gacc[:], ACTF.Sigmoid)
        nc.gpsimd.dma_start(gp_d.ap(), gpo[:])

        # ---- h^T resident [p, hc*2048 + bl]: 16 partition-half DMAs across queues ----
        hT = misc.tile([128, HC * BL], F32R, name="hT", tag="hT")
        for hc in range(HC):
            for phalf in range(2):
                nc.sync.dma_start(
                    hT[phalf * 64:(phalf + 1) * 64, hc * BL:(hc + 1) * BL],
                    bass.AP(
                        hT_d, (hc * 128 + phalf * 64) * BL, [[BL, 64], [1, BL]]
                    ).bitcast(F32R),
                )

        # ---- acc tiles ----
        accs = [accp.tile([128, VS], F32, name=f"acc{i}", tag="acc") for i in range(NBLC)]


        # ---- main gated expert matmuls, two W-streaming passes over vocab ----
        written = set()
        for ci, (off, n) in enumerate(VCS):
            subs = [(so, min(512, n - so)) for so in range(0, n, 512)]
            for e in range(E):
                tiles = work[e]
                if not tiles:
                    continue
                wts = []
                for hc in range(HC):
                    wt = wpool.tile([128, 512], F32R, name=f"wt{ci}_{e}_{hc}", tag="wt")
                    nc.sync.dma_start(
                        wt[:, 0:n],
                        bass.AP(
                            wt_d, e * H * VS + hc * 128 * VS + off, [[VS, 128], [1, n]]
                        ).bitcast(F32R),
                    )
                    wts.append(wt)
                if with_bias:
                    cbt = cbpool.tile([1, 512], F32R, name=f"cbt{ci}_{e}", tag="cbt")
                    nc.sync.dma_start(
                        cbt[0:1, 0:n],
                        bass.AP(cb_d, e * VS + off, [[1, 1], [1, n]]).bitcast(F32R),
                    )
                for (blc, d0, d1) in tiles:
                    for (so, sn) in subs:
                        pt = ps.tile([128, 512], F32, name=f"pt{ci}_{e}_{blc}_{so}", tag="ps")
                        for hc in range(HC):
                            nc.tensor.matmul(
                                pt[:, 0:sn],
                                hT[:, hc * BL + blc * 128: hc * BL + blc * 128 + 128],
                                wts[hc][:, so:so + sn],
                                start=(hc == 0),
                                stop=(hc == HC - 1 and not with_bias),
                            )
                        if with_bias:
                            nc.tensor.matmul(
                                pt[:, 0:sn],
                                onesr[0:1, 0:128],
                                cbt[0:1, so:so + sn],
                                start=False,
                                stop=True,
                            )
                        for phh, active in ((0, d0), (1, d1)):
                            if not active:
                                continue
                            s = 2 * blc + phh
                            psrc = pt[phh * 64:(phh + 1) * 64, 0:sn]
                            gsc = grep[phh * 64:(phh + 1) * 64, blc * E + e: blc * E + e + 1]
                            adst = accs[blc][phh * 64:(phh + 1) * 64, off + so:off + so + sn]
                            if (s, off + so) in written:
                                nc.vector.scalar_tensor_tensor(
                                    adst, psrc, gsc, adst, op0=ALU.mult, op1=ALU.add
                                )
                            else:
                                nc.scalar.activation(adst, psrc, ACTF.Copy, scale=gsc)
                                written.add((s, off + so))
            for blc in range(NBLC):
                nc.scalar.dma_start(
                    comp_d.ap()[blc * 128:(blc + 1) * 128, off:off + n],
                    accs[blc][:, off:off + n],
                )

    nc.compile()
    return nc


def _get_program(work, with_bias):
    key = (work, with_bias)
    if key not in _cache:
        _cache[key] = _build(work, with_bias)
    return _cache[key]


def kernel(**inputs):
    f = lambda k: np.ascontiguousarray(np.asarray(inputs[k], dtype=np.float32))
    q = f("query_repr")
    h = f("h")
    w1 = f("gate_w1")
    b1 = f("gate_b1")
    w2 = f("gate_w2")
    b2 = f("gate_b2")
    gw = f("gauss_w")
    gb = f("gauss_b")
    cw = f("comp_w")
    cb = f("comp_b")

    # host routing (work-list only; gate values come from the device)
    hid = np.maximum(q @ w1 + b1, 0.0)
    logits = hid @ w2 + b2
    order = np.argsort(-logits, axis=1, kind="stable")
    srt = np.take_along_axis(logits, order, axis=1)
    tie_risk = np.min(srt[:, 1] - srt[:, 2]) < 1e-5 or np.min(srt[:, 0] - srt[:, 1]) < 1e-5
    mode = os.environ.get("MOE_KERNEL_MODE", "auto")
    if mode == "dense" or (mode == "auto" and tie_risk):
        work = _dense_work()
    else:
        work = _route_work(order[:, :2])
    with_bias = bool(np.any(cb != 0.0))

    nc = _get_program(work, with_bias)

    hT = np.ascontiguousarray(h.reshape(BL, H).T)
    shared = {
        "hT_d": hT,
        "qT_d": np.ascontiguousarray(q.T),
        "hlT_d": np.ascontiguousarray(h[:, -1].T),
        "w1_d": w1,
        "b1_d": b1,
        "w2_d": w2,
        "b2_d": b2,
        "gwT_d": np.ascontiguousarray(gw.transpose(2, 0, 1).reshape(H, E * OUT)),
        "gb_d": gb.reshape(E * OUT),
        "ones_d": np.ones(128, np.float32),
    }
    in_maps = []
    for c in range(NCORES):
        sl = slice(c * VS, (c + 1) * VS)
        in_maps.append(
            dict(
                shared,
                wt_d=np.ascontiguousarray(cw[:, sl, :].transpose(0, 2, 1)),
                cb_d=np.ascontiguousarray(cb[:, sl]),
            )
        )

    trace = os.environ.get("MOE_KERNEL_TRACE") == "1"
    if trace:
        try:
            import sys, types

            if "antenv.axon_hooks" not in sys.modules:
                import antenv  # noqa: F401

                mod = types.ModuleType("antenv.axon_hooks")
                mod._hook = None
                mod.set_axon_ntff_profile_hook = lambda hk: setattr(mod, "_hook", hk)
                mod.get_axon_ntff_profile_hook = lambda: mod._hook
                sys.modules["antenv.axon_hooks"] = mod
                from trn_agent_boot.trn_boot import _ntff_profile_via_ctypes

                mod._hook = _ntff_profile_via_ctypes("/opt/axon/libaxon_pjrt.so")
        except Exception as exc:  # pragma: no cover
            print(f"trace hook install failed: {exc}")

    res = bass_utils.run_bass_kernel_spmd(
        nc, in_maps, core_ids=list(range(NCORES)), trace=trace
    )
    if trace and res.exec_time_ns is not None:
        print(f"HW exec time: {res.exec_time_ns} ns")

    comp = np.concatenate(
        [res.results[c]["comp_d"].reshape(B, L, VS) for c in range(NCORES)], axis=2
    )
    gp = res.results[0]["gp_d"]
    loss = np.float32(res.results[0]["loss_d"][0, 0])
    return gp, comp, loss


# revision 19
# speedup vs baseline: 1.0305x; 1.0066x over previous
"""Trainium2 Bass kernel for nn_CPL_MoE (query-guided MoE: gating MLP -> top-2
softmax gates -> gated expert matmul accumulation + gauss head + balance loss).

Strategy: vocab/tensor-parallel across the 8 NeuronCores (each core owns a
V/8 = 1500 column slice of comp_w / comp_b / comp output; gating replicated).
The big expert matmuls run in float32r (full-rate fp32 on the PE array) and
exploit the top-2 routing sparsity: routing is computed on the host from the
gating inputs, and the program only emits matmul work for (expert, sample)
pairs that the top-2 selection actually activates.  The gate VALUES applied on
device come from the on-device gating computation, so results match the dense
computation exactly (skipped blocks would have been multiplied by gates == 0).

Self-contained: only needs numpy + the concourse (Bass) stack available in the
container.
"""

import os
import numpy as np
from contextlib import ExitStack

import concourse.bass as bass
import concourse.tile as tile
import concourse.bacc as bacc
from concourse import mybir, bass_utils

F32 = mybir.dt.float32
F32R = mybir.dt.float32r
AX = mybir.AxisListType
ALU = mybir.AluOpType
ACTF = mybir.ActivationFunctionType

# problem shapes (hardcoded per spec)
B, L, H, E, V = 32, 64, 1024, 8, 12000
OUT = 16
HID = 512
COEF = 0.1
NCORES = 8
VS = V // NCORES            # 1500 vocab columns per core
BL = B * L                  # 2048
HC = H // 128               # 8 contraction chunks
NBLC = BL // 128            # 16 row tiles of comp
VCS = [(0, 512), (512, 512), (1024, VS - 1024)]   # psum-bank sized v chunks

_cache: dict = {}


def _dense_work():
    tiles = tuple((blc, True, True) for blc in range(B // 2))
    return tuple(tiles for _ in range(E))


def _route_work(top2):
    """top2: [B, 2] expert ids per sample -> per-expert (blc, drain0, drain1) tiles.

    A row tile blc covers samples (2*blc, 2*blc+1).  An expert computes a tile
    when either sample routes to it; only the active halves are drained into
    the accumulator (the other half's gate is 0 anyway)."""
    sel = [set() for _ in range(E)]
    for s in range(B):
        for e in top2[s]:
            sel[int(e)].add(s)
    work = []
    for e in range(E):
        tiles = []
        for blc in range(B // 2):
            d0 = 2 * blc in sel[e]
            d1 = 2 * blc + 1 in sel[e]
            if d0 or d1:
                tiles.append((blc, d0, d1))
        work.append(tuple(tiles))
    return tuple(work)


def _build(work, with_bias):
    nc = bacc.Bacc("TRN2", target_bir_lowering=False, debug=False)

    # ---- DRAM I/O ----
    hT_d = nc.dram_tensor("hT_d", [H, BL], F32, kind="ExternalInput")
    qT_d = nc.dram_tensor("qT_d", [H, B], F32, kind="ExternalInput")
    hlT_d = nc.dram_tensor("hlT_d", [H, B], F32, kind="ExternalInput")
    w1_d = nc.dram_tensor("w1_d", [H, HID], F32, kind="ExternalInput")
    b1_d = nc.dram_tensor("b1_d", [HID], F32, kind="ExternalInput")
    w2_d = nc.dram_tensor("w2_d", [HID, E], F32, kind="ExternalInput")
    b2_d = nc.dram_tensor("b2_d", [E], F32, kind="ExternalInput")
    gwT_d = nc.dram_tensor("gwT_d", [H, E * OUT], F32, kind="ExternalInput")
    gb_d = nc.dram_tensor("gb_d", [E * OUT], F32, kind="ExternalInput")
    wt_d = nc.dram_tensor("wt_d", [E, H, VS], F32, kind="ExternalInput")
    ones_d = nc.dram_tensor("ones_d", [128], F32, kind="ExternalInput")
    cb_d = nc.dram_tensor("cb_d", [E, VS], F32, kind="ExternalInput")

    comp_d = nc.dram_tensor("comp_d", [BL, VS], F32, kind="ExternalOutput")
    gp_d = nc.dram_tensor("gp_d", [B, OUT], F32, kind="ExternalOutput")
    loss_d = nc.dram_tensor("loss_d", [1, 1], F32, kind="ExternalOutput")
    gsc_d = nc.dram_tensor("gsc_d", [B, E], F32, kind="Internal")

    with tile.TileContext(nc) as tc, ExitStack() as ctx:
        misc = ctx.enter_context(tc.tile_pool(name="misc", bufs=1))
        accp = ctx.enter_context(tc.tile_pool(name="accp", bufs=NBLC))
        wpool = ctx.enter_context(tc.tile_pool(name="wpool", bufs=13))
        w1pool = ctx.enter_context(tc.tile_pool(name="w1pool", bufs=4))
        cbpool = ctx.enter_context(tc.tile_pool(name="cbpool", bufs=2))
        ps = ctx.enter_context(tc.tile_pool(name="ps", bufs=8, space="PSUM"))

        def mtile(shape, name):
            return misc.tile(shape, F32, name=name, tag=name)

        # ---- constants ----
        ones = mtile([1, 128], "ones")
        nc.vector.memset(ones[:], 1.0)
        onesr = misc.tile([1, 128], F32R, name="onesr", tag="onesr")
        nc.sync.dma_start(onesr[0:1, :], bass.AP(ones_d, 0, [[1, 1], [1, 128]]).bitcast(F32R))
        onescol = mtile([32, 1], "onescol")
        nc.vector.memset(onescol[:], 1.0)

        # ---- gating input DMAs ----
        qT = mtile([128, HC * B], "qT")            # [p, hc*32+b]
        nc.sync.dma_start(qT[:], bass.AP(qT_d, 0, [[B, 128], [128 * B, HC], [1, B]]))
        hlT = mtile([128, HC * B], "hlT")
        nc.sync.dma_start(hlT[:], bass.AP(hlT_d, 0, [[B, 128], [128 * B, HC], [1, B]]))
        b1sb = mtile([128, 4], "b1sb")             # [p, hidc]
        nc.sync.dma_start(b1sb[:], bass.AP(b1_d, 0, [[1, 128], [128, 4]]))
        w2sb = mtile([128, 4 * E], "w2sb")         # [p, hidc*8+e]
        nc.sync.dma_start(w2sb[:], bass.AP(w2_d, 0, [[E, 128], [128 * E, 4], [1, E]]))
        b2sb = mtile([1, E], "b2sb")
        nc.sync.dma_start(b2sb[:], bass.AP(b2_d, 0, [[1, 1], [1, E]]))
        gbsb = mtile([1, E * OUT], "gbsb")
        nc.sync.dma_start(gbsb[:], bass.AP(gb_d, 0, [[1, 1], [1, E * OUT]]))

        # ---- gating MLP: hidT [hid(4x128 part chunks), b] = relu(w1.T @ q + b1) ----
        hidT = mtile([128, 4 * B], "hidT")         # [p, hidc*32+b]
        hps = [ps.tile([128, B], F32, name=f"hps{i}", tag="ps") for i in range(4)]
        for hc in range(HC):
            w1c = w1pool.tile([128, HID], F32, name=f"w1c{hc}", tag="w1c")
            nc.scalar.dma_start(
                w1c[:], bass.AP(w1_d, hc * 128 * HID, [[HID, 128], [1, HID]])
            )
            for hidc in range(4):
                nc.tensor.matmul(
                    hps[hidc][:],
                    w1c[:, hidc * 128:(hidc + 1) * 128],
                    qT[:, hc * B:(hc + 1) * B],
                    start=(hc == 0),
                    stop=(hc == HC - 1),
                )
        for hidc in range(4):
            nc.scalar.activation(
                hidT[:, hidc * B:(hidc + 1) * B], hps[hidc][:], ACTF.Relu,
                bias=b1sb[:, hidc:hidc + 1], scale=1.0,
            )

        # ---- logits [b=32, e=8] ----
        lps = ps.tile([B, E], F32, name="lps", tag="ps")
        for hidc in range(4):
            nc.tensor.matmul(
                lps[:], hidT[:, hidc * B:(hidc + 1) * B], w2sb[:, hidc * E:(hidc + 1) * E],
                start=(hidc == 0), stop=False,
            )
        nc.tensor.matmul(lps[:], ones[0:1, 0:B], b2sb[0:1, :], start=False, stop=True)
        lsb = mtile([B, E], "lsb")
        nc.scalar.copy(lsb[:], lps[:])

        # ---- top-2 softmax gates [32, 8] ----
        m1 = mtile([B, 1], "m1")
        negm1 = mtile([B, 1], "negm1")
        e1 = mtile([B, E], "e1")
        mask1 = mtile([B, E], "mask1")
        lmsk = mtile([B, E], "lmsk")
        m2 = mtile([B, 1], "m2")
        mask2 = mtile([B, E], "mask2")
        e2m = mtile([B, 1], "e2m")
        den = mtile([B, 1], "den")
        rec = mtile([B, 1], "rec")
        gates = mtile([B, E], "gates")
        nc.vector.tensor_reduce(m1[:], lsb[:], axis=AX.X, op=ALU.max)
        nc.vector.tensor_scalar_mul(negm1[:], m1[:], -1.0)
        nc.scalar.activation(e1[:], lsb[:], ACTF.Exp, bias=negm1[:], scale=1.0)
        nc.vector.tensor_scalar(mask1[:], lsb[:], m1[:], None, op0=ALU.is_ge)
        nc.vector.scalar_tensor_tensor(lmsk[:], mask1[:], -1e30, lsb[:], op0=ALU.mult, op1=ALU.add)
        nc.vector.tensor_reduce(m2[:], lmsk[:], axis=AX.X, op=ALU.max)
        nc.vector.tensor_scalar(mask2[:], lsb[:], m2[:], None, op0=ALU.is_ge)
        nc.scalar.activation(e2m[:], m2[:], ACTF.Exp, bias=negm1[:], scale=1.0)
        nc.vector.tensor_scalar_add(den[:], e2m[:], 1.0)
        nc.vector.reciprocal(rec[:], den[:])
        nc.vector.scalar_tensor_tensor(gates[:], e1[:], rec[:], mask2[:], op0=ALU.mult, op1=ALU.mult)

        # ---- replicate gates along 64 l-rows: grep[p, blc*8+e] = gates[2blc+p//64, e] ----
        grep = mtile([128, NBLC * E], "grep")
        nc.gpsimd.dma_start(gsc_d.ap(), gates[:])
        for phh in range(2):
            nc.gpsimd.dma_start(
                grep[phh * 64:(phh + 1) * 64, :],
                bass.AP(gsc_d, phh * E, [[0, 64], [2 * E, NBLC], [1, E]]),
            )

        # ---- moe balance loss ----
        imp_ps = ps.tile([E, 1], F32, name="imp_ps", tag="ps")
        nc.tensor.matmul(imp_ps[:], gates[:], onescol[:, :], start=True, stop=True)
        imp = mtile([E, 1], "imp")
        nc.scalar.copy(imp[:], imp_ps[:])
        s1_ps = ps.tile([1, 1], F32, name="s1_ps", tag="ps")
        nc.tensor.matmul(s1_ps[:], imp[:], onescol[0:E, :], start=True, stop=True)
        s2_ps = ps.tile([1, 1], F32, name="s2_ps", tag="ps")
        nc.tensor.matmul(s2_ps[:], imp[:], imp[:], start=True, stop=True)
        s1 = mtile([1, 1], "s1")
        s2 = mtile([1, 1], "s2")
        nc.scalar.copy(s1[:], s1_ps[:])
        nc.scalar.copy(s2[:], s2_ps[:])
        mean = mtile([1, 1], "mean")
        nc.scalar.mul(mean[:], s1[:], 1.0 / E)
        meps = mtile([1, 1], "meps")
        nc.vector.tensor_scalar_add(meps[:], mean[:], 1e-10)
        rmean = mtile([1, 1], "rmean")
        nc.vector.reciprocal(rmean[:], meps[:])
        s1sq = mtile([1, 1], "s1sq")
        nc.vector.tensor_mul(s1sq[:], s1[:], s1[:])
        va = mtile([1, 1], "va")
        nc.scalar.mul(va[:], s2[:], 1.0 / (E - 1))
        vb = mtile([1, 1], "vb")
        nc.scalar.mul(vb[:], s1sq[:], 1.0 / (E * (E - 1)))
        var = mtile([1, 1], "var")
        nc.vector.tensor_sub(var[:], va[:], vb[:])
        std = mtile([1, 1], "std")
        nc.scalar.sqrt(std[:], var[:])
        lossv = mtile([1, 1], "lossv")
        nc.vector.tensor_mul(lossv[:], std[:], rmean[:])
        losso = mtile([1, 1], "losso")
        nc.scalar.mul(losso[:], lossv[:], COEF)
        nc.gpsimd.dma_start(loss_d.ap(), losso[:])

        # ---- gauss head: gp = sigmoid(sum_e g[b,e] * (h_last @ gauss_w[e].T + gauss_b[e])) ----
        gps_ = ps.tile([B, E * OUT], F32, name="gps_", tag="ps")
        for half in range(2):
            gwc = w1pool.tile([128, 4 * E * OUT], F32, name=f"gwc{half}", tag="w1c")
            nc.scalar.dma_start(
                gwc[:],
                bass.AP(
                    gwT_d, half * 4 * 128 * E * OUT,
                    [[E * OUT, 128], [128 * E * OUT, 4], [1, E * OUT]],
                ),
            )
            for k in range(4):
                hc = half * 4 + k
                nc.tensor.matmul(
                    gps_[:], hlT[:, hc * B:(hc + 1) * B], gwc[:, k * E * OUT:(k + 1) * E * OUT],
                    start=(hc == 0), stop=False,
                )
        nc.tensor.matmul(gps_[:], ones[0:1, 0:B], gbsb[0:1, :], start=False, stop=True)
        gacc = mtile([B, OUT], "gacc")
        nc.vector.tensor_scalar(gacc[:], gps_[:, 0:OUT], gates[:, 0:1], None, op0=ALU.mult)
        for e in range(1, E):
            nc.vector.scalar_tensor_tensor(
                gacc[:], gps_[:, e * OUT:(e + 1) * OUT], gates[:, e:e + 1], gacc[:],
                op0=ALU.mult, op1=ALU.add,
            )
        gpo = mtile([B, OUT], "gpo")
        nc.scalar.activation(gpo[:], gacc[:], ACTF.Sigmoid)
        nc.gpsimd.dma_start(gp_d.ap(), gpo[:])

        # ---- h^T resident [p, hc*2048 + bl]: 16 partition-half DMAs across queues ----
        hT = misc.tile([128, HC * BL], F32R, name="hT", tag="hT")
        for hc in range(HC):
            for phalf in range(2):
                nc.sync.dma_start(
                    hT[phalf * 64:(phalf + 1) * 64, hc * BL:(hc + 1) * BL],
                    bass.AP(
                        hT_d, (hc * 128 + phalf * 64) * BL, [[BL, 64], [1, BL]]
                    ).bitcast(F32R),
                )

        # ---- acc tiles ----
        accs = [accp.tile([128, VS], F32, name=f"acc{i}", tag="acc") for i in range(NBLC)]


        # ---- main gated expert matmuls, two W-streaming passes over vocab ----
        written = set()
        for ci, (off, n) in enumerate(VCS):
            subs = [(so, min(512, n - so)) for so in range(0, n, 512)]
            for e in range(E):
                tiles = work[e]
                if not tiles:
                    continue
                wts = []
                for hc in range(HC):
                    wt = wpool.tile([128, 512], F32R, name=f"wt{ci}_{e}_{hc}", tag="wt")
                    nc.sync.dma_start(
                        wt[:, 0:n],
                        bass.AP(
                            wt_d, e * H * VS + hc * 128 * VS + off, [[VS, 128], [1, n]]
                        ).bitcast(F32R),
                    )
                    wts.append(wt)
                if with_bias:
                    cbt = cbpool.tile([1, 512], F32R, name=f"cbt{ci}_{e}", tag="cbt")
                    nc.sync.dma_start(
                        cbt[0:1, 0:n],
                        bass.AP(cb_d, e * VS + off, [[1, 1], [1, n]]).bitcast(F32R),
                    )
                for (blc, d0, d1) in tiles:
                    for (so, sn) in subs:
                        pt = ps.tile([128, 512], F32, name=f"pt{ci}_{e}_{blc}_{so}", tag="ps")
                        for hc in range(HC):
                            nc.tensor.matmul(
                                pt[:, 0:sn],
                                hT[:, hc * BL + blc * 128: hc * BL + blc * 128 + 128],
                                wts[hc][:, so:so + sn],
                                start=(hc == 0),
                                stop=(hc == HC - 1 and not with_bias),
                            )
                        if with_bias:
                            nc.tensor.matmul(
                                pt[:, 0:sn],
                                onesr[0:1, 0:128],
                                cbt[0:1, so:so + sn],
                                start=False,
                                stop=True,
                            )
                        for phh, active in ((0, d0), (1, d1)):
                            if not active:
                                continue
                            s = 2 * blc + phh
                            psrc = pt[phh * 64:(phh + 1) * 64, 0:sn]
                            gsc = grep[phh * 64:(phh + 1) * 64, blc * E + e: blc * E + e + 1]
                            adst = accs[blc][phh * 64:(phh + 1) * 64, off + so:off + so + sn]
                            if (s, off + so) in written:
                                nc.vector.scalar_tensor_tensor(
                                    adst, psrc, gsc, adst, op0=ALU.mult, op1=ALU.add
                                )
                            else:
                                nc.scalar.activation(adst, psrc, ACTF.Copy, scale=gsc)
                                written.add((s, off + so))
            for blc in range(NBLC):
                nc.scalar.dma_start(
                    comp_d.ap()[blc * 128:(blc + 1) * 128, off:off + n],
                    accs[blc][:, off:off + n],
                )

    nc.compile()
    return nc


def _get_program(work, with_bias):
    key = (work, with_bias)
    if key not in _cache:
        _cache[key] = _build(work, with_bias)
    return _cache[key]


def kernel(**inputs):
    f = lambda k: np.ascontiguousarray(np.asarray(inputs[k], dtype=np.float32))
    q = f("query_repr")
    h = f("h")
    w1 = f("gate_w1")
    b1 = f("gate_b1")
    w2 = f("gate_w2")
    b2 = f("gate_b2")
    gw = f("gauss_w")
    gb = f("gauss_b")
    cw = f("comp_w")
    cb = f("comp_b")

    # host routing (work-list only; gate values come from the device)
    hid = np.maximum(q @ w1 + b1, 0.0)
    logits = hid @ w2 + b2
    order = np.argsort(-logits, axis=1, kind="stable")
    srt = np.take_along_axis(logits, order, axis=1)
    tie_risk = np.min(srt[:, 1] - srt[:, 2]) < 1e-5 or np.min(srt[:, 0] - srt[:, 1]) < 1e-5
    mode = os.environ.get("MOE_KERNEL_MODE", "auto")
    if mode == "dense" or (mode == "auto" and tie_risk):
        work = _dense_work()
    else:
        work = _route_work(order[:, :2])
    with_bias = bool(np.any(cb != 0.0))

    nc = _get_program(work, with_bias)

    hT = np.ascontiguousarray(h.reshape(BL, H).T)
    shared = {
        "hT_d": hT,
        "qT_d": np.ascontiguousarray(q.T),
        "hlT_d": np.ascontiguousarray(h[:, -1].T),
        "w1_d": w1,
        "b1_d": b1,
        "w2_d": w2,
        "b2_d": b2,
        "gwT_d": np.ascontiguousarray(gw.transpose(2, 0, 1).reshape(H, E * OUT)),
        "gb_d": gb.reshape(E * OUT),
        "ones_d": np.ones(128, np.float32),
    }
    in_maps = []
    for c in range(NCORES):
        sl = slice(c * VS, (c + 1) * VS)
        in_maps.append(
            dict(
                shared,
                wt_d=np.ascontiguousarray(cw[:, sl, :].transpose(0, 2, 1)),
                cb_d=np.ascontiguousarray(cb[:, sl]),
            )
        )

    trace = os.environ.get("MOE_KERNEL_TRACE") == "1"
    if trace:
        try:
            import sys, types

            if "antenv.axon_hooks" not in sys.modules:
                import antenv  # noqa: F401

                mod = types.ModuleType("antenv.axon_hooks")
                mod._hook = None
                mod.set_axon_ntff_profile_hook = lambda hk: setattr(mod, "_hook", hk)
                mod.get_axon_ntff_profile_hook = lambda: mod._hook
                sys.modules["antenv.axon_hooks"] = mod
                from trn_agent_boot.trn_boot import _ntff_profile_via_ctypes

                mod._hook = _ntff_profile_via_ctypes("/opt/axon/libaxon_pjrt.so")
        except Exception as exc:  # pragma: no cover
            print(f"trace hook install failed: {exc}")

    res = bass_utils.run_bass_kernel_spmd(
        nc, in_maps, core_ids=list(range(NCORES)), trace=trace
    )
    if trace and res.exec_time_ns is not None:
        print(f"HW exec time: {res.exec_time_ns} ns")

    comp = np.concatenate(
        [res.results[c]["comp_d"].reshape(B, L, VS) for c in range(NCORES)], axis=2
    )
    gp = res.results[0]["gp_d"]
    loss = np.float32(res.results[0]["loss_d"][0, 0])
    return gp, comp, loss
